# revision 16
# baseline (speedup 1.0000x reference)
"""Trainium2 Bass kernel for nn_CrossAttnVDTBlock (B=2,T=8,N=256,D=768,H=12,DFF=3072).

Sharding: 8 NeuronCores = 2 batch-groups x 4 frame-pair shards. Core c serves
batch c//4 and owns query frames (g, 7-g), g=c%4 (512 tokens, host-permuted to
the front). v1 is collective-free: each core redundantly computes the
cross-attention stage and the self-attention K/V for its whole batch (2048
tokens), then self-attention scores/AV and the MLP only for its own 512 query
tokens. Frame-causal masks are folded into the score matmuls via 8 augmented
contraction rows (K side: one-hot frame id; Q side: -30000*[f > frame(q)]), so
masking costs no elementwise work. The host pre-fuses c_wo@w_fc1 and s_wo@w_fc2
(no nonlinearity between them), folds hd^-0.5 into wq, and casts weights to
bf16. Matmuls run bf16 (fp32 PSUM); the residual stream stays fp32 on-chip.
Activations are feature-major [D, tokens] throughout - no device transposes.
"""

import contextlib

import numpy as np
import ml_dtypes

import concourse.bass as bass
import concourse.mybir as mybir
import concourse.tile as tile
from concourse import bacc
from concourse.bass import ts
from concourse.bass_utils import run_bass_kernel_spmd

F32 = mybir.dt.float32
F32R = mybir.dt.float32r
BF16 = mybir.dt.bfloat16
AF = mybir.ActivationFunctionType
ALU = mybir.AluOpType

B, T, NT, D, H, DFF = 2, 8, 256, 768, 12, 3072
hd = D // H          # 64
S = T * NT           # 2048
P = 128
KD = D // P          # 6 din tiles
KF = DFF // P        # 24 dff tiles
NEG = -30000.0
EPS = 1e-6
NCORE = 8
OWN = 512
NCH = S // 512       # 4 column chunks of 512

_bf = ml_dtypes.bfloat16


def _ln(tc, nc, ctx, getx, ncols, rb, mb, ones, sc1_ap, sh_ap, out_xt):
    """LayerNorm over features of feature-major x (via getx(j, chunk) -> AP
    [128,512]), optionally adaLN-modulated; writes bf16 out_xt [128,KD,ncols].
    rb/mb: [128,>=ncols] bf16 broadcast scratch."""
    nchunks = ncols // 512
    with tc.tile_pool(name="lnp", bufs=2, space="PSUM") as pp, \
            tc.tile_pool(name="lns", bufs=1) as sp, \
            tc.tile_pool(name="lnt", bufs=3) as tp:
        for c in range(nchunks):
            cs = ts(c, 512)
            ps_s = pp.tile([1, 512], F32, tag="ln_s")
            ps_q = pp.tile([1, 512], F32, tag="ln_q")
            for j in range(KD):
                xj = getx(j, c)
                xb = tp.tile([P, 512], BF16, tag="xb")
                nc.vector.tensor_copy(xb[:], xj)
                xsq = tp.tile([P, 512], BF16, tag="xsq")
                nc.vector.tensor_tensor(xsq[:], xj, xj, ALU.mult)
                nc.tensor.matmul(ps_s[:], ones[:], xb[:],
                                 start=(j == 0), stop=(j == KD - 1))
                nc.tensor.matmul(ps_q[:], ones[:], xsq[:],
                                 start=(j == 0), stop=(j == KD - 1))
            nc.vector.tensor_scalar_mul(ps_s[:], ps_s[:], -1.0 / D)
            nc.vector.tensor_scalar(ps_q[:], ps_q[:], 1.0 / D, EPS, ALU.mult,
                                    ALU.add)
            mu2 = sp.tile([1, 512], F32, tag="mu2", name=f"mu2_{c}")
            nc.scalar.activation(mu2[:], ps_s[:], AF.Square)
            nc.vector.tensor_tensor(ps_q[:], ps_q[:], mu2[:], ALU.subtract)
            nc.scalar.activation(ps_q[:], ps_q[:], AF.Sqrt)
            rr = sp.tile([1, 512], F32, tag="rr", name=f"rr_{c}")
            nc.vector.reciprocal(rr[:], ps_q[:])
            nm = sp.tile([1, 512], F32, tag="nm", name=f"nm_{c}")
            nc.vector.tensor_tensor(nm[:], ps_s[:], rr[:], ALU.mult)
            rrb = sp.tile([1, 512], BF16, tag="rrb", name=f"rrb_{c}")
            nc.vector.tensor_copy(rrb[:], rr[:])
            nmb = sp.tile([1, 512], BF16, tag="nmb", name=f"nmb_{c}")
            nc.vector.tensor_copy(nmb[:], nm[:])
            nc.gpsimd.partition_broadcast(rb[:, cs], rrb[:])
            nc.gpsimd.partition_broadcast(mb[:, cs], nmb[:])
        for j in range(KD):
            for c in range(nchunks):
                cs = ts(c, 512)
                t1 = tp.tile([P, 512], F32, tag="lnt1")
                nc.vector.tensor_tensor(t1[:], getx(j, c), rb[:, cs], ALU.mult)
                if sc1_ap is None:
                    nc.vector.tensor_tensor(out_xt[:, j, cs], t1[:], mb[:, cs],
                                            ALU.add)
                else:
                    nc.vector.tensor_tensor(t1[:], t1[:], mb[:, cs], ALU.add)
                    nc.vector.tensor_scalar(out_xt[:, j, cs], t1[:],
                                            sc1_ap[:, j, None],
                                            sh_ap[:, j, None],
                                            ALU.mult, ALU.add)


def _emit_kernel(tc, io):
    nc = tc.nc
    st = contextlib.ExitStack()
    pool = lambda **kw: st.enter_context(tc.tile_pool(**kw))

    persist = pool(name="persist", bufs=1)
    tmp = pool(name="tmp", bufs=2)
    small = pool(name="small", bufs=2)

    # ---------------- persistent state ----------------
    x_own = persist.tile([P, KD, OWN], F32, tag="x_own")
    ones_r = persist.tile([P, 1], BF16, tag="ones")
    nc.any.memset(ones_r[:], 1.0)
    rb = persist.tile([P, S], BF16, tag="rb")
    mbb = persist.tile([P, S], BF16, tag="mbb")
    mods = persist.tile([P, 36], F32, tag="mods")
    qmask = persist.tile([8, S], BF16, tag="qmask")
    khot = persist.tile([8, S], BF16, tag="khot")
    zhot = persist.tile([8, 8], BF16, tag="zhot")
    zb = persist.tile([P, KD, 8], BF16, tag="zb")
    u2 = persist.tile([P, KD, OWN], BF16, tag="u2")

    nc.sync.dma_start(x_own[:],
                      io["xT"][:, 0:OWN].rearrange("(j p) t -> p j t", p=P))
    nc.sync.dma_start(qmask[:], io["qmask"][:])
    nc.sync.dma_start(khot[:], io["khot"][:])
    nc.sync.dma_start(zhot[:], io["zhot"][:])
    nc.sync.dma_start(zb[:], io["zT"].rearrange("(j p) t -> p j t", p=P))

    bias = {}
    for nm_ in ("cbq", "cbk", "bc", "sbq", "sbk", "bs", "mb2"):
        bt = persist.tile([P, KD], F32, tag="b_" + nm_)
        nc.sync.dma_start(bt[:], io[nm_][:])
        bias[nm_] = bt
    mb1 = persist.tile([P, KF], F32, tag="b_mb1")
    nc.sync.dma_start(mb1[:], io["mb1"][:])
    vrow_c = persist.tile([1, H * 65], BF16, tag="vrow_c")
    nc.sync.dma_start(vrow_c[:], io["cbv_row"][:])
    vrow_s = persist.tile([1, H * 65], BF16, tag="vrow_s")
    nc.sync.dma_start(vrow_s[:], io["sbv_row"][:])
    vb_c = persist.tile([P, H * 65], BF16, tag="vb_c")
    nc.gpsimd.partition_broadcast(vb_c[:], vrow_c[:])
    vb_s = persist.tile([P, H * 65], BF16, tag="vb_s")
    nc.gpsimd.partition_broadcast(vb_s[:], vrow_s[:])

    def modap(i):  # chunk i of (sh_s, sc_s, g_s, sh_m, sc_m, g_m)
        return mods[:, ts(i, KD)]

    # ---------------- mods = silu(t) @ w_ada + b_ada ----------------
    with tc.tile_pool(name="ada", bufs=1) as ap, \
            tc.tile_pool(name="adap", bufs=1, space="PSUM") as app:
        wada = ap.tile([P, KD, 6 * D], BF16)
        nc.sync.dma_start(wada[:],
                          io["w_ada"].rearrange("(j p) o -> p j o", p=P))
        tsil = ap.tile([P, KD], BF16)
        nc.sync.dma_start(tsil[:], io["tsil"][:])
        tsig = ap.tile([P, KD], BF16)
        nc.scalar.activation(tsig[:], tsil[:], AF.Silu)
        bada = ap.tile([P, 36], F32)
        nc.sync.dma_start(bada[:], io["bada"][:])
        ps_m = app.tile([P, 36], F32)
        for m in range(36):
            for j in range(KD):
                nc.tensor.matmul(ps_m[:, m:m + 1], wada[:, j, ts(m, P)],
                                 tsig[:, j, None], start=(j == 0),
                                 stop=(j == KD - 1))
        nc.vector.tensor_tensor(mods[:], ps_m[:], bada[:], ALU.add)

    # =========== stages 1+2 need the full-batch residual ===========
    with tc.tile_pool(name="bigx", bufs=1) as bigp:
        xst = contextlib.ExitStack()
        xrp = xst.enter_context(tc.tile_pool(name="xrestp", bufs=1))
        x_rest = xrp.tile([P, KD, S - OWN], F32, tag="x_rest")
        nc.sync.dma_start(
            x_rest[:], io["xT"][:, OWN:S].rearrange("(j p) t -> p j t", p=P))
        xt = bigp.tile([P, KD, S], BF16, tag="xt")  # normalized activations

        def getx(j, c):
            if c == 0:
                return x_own[:, j, :]
            return x_rest[:, j, ts(c - 1, 512)]

        # ---------------- stage 1: cross attention ----------------
        _ln(tc, nc, st, getx, S, rb, mbb, ones_r, None, None, xt)

        with tc.tile_pool(name="s1w", bufs=2) as wp, \
                tc.tile_pool(name="s1", bufs=1) as s1p, \
                tc.tile_pool(name="s1q", bufs=2) as qcp, \
                tc.tile_pool(name="s1mm", bufs=2, space="PSUM") as pmm, \
                tc.tile_pool(name="s1sc", bufs=2, space="PSUM") as psc, \
                tc.tile_pool(name="s1av", bufs=2, space="PSUM") as pav:
            wk = wp.tile([P, KD, D], BF16, tag="w")
            nc.sync.dma_start(wk[:],
                              io["c_wk"].rearrange("(j p) o -> p j o", p=P))
            wv = wp.tile([P, KD, D], BF16, tag="w")
            nc.sync.dma_start(wv[:],
                              io["c_wv"].rearrange("(j p) o -> p j o", p=P))
            # kz feature-major [72, 8] per head; vz grouped [32, 3, 4, 65]
            kz = s1p.tile([72, 8 * H], BF16, tag="kz")
            vz = s1p.tile([8, H, 65], BF16, tag="vz")
            for j in range(KD):
                ps = pmm.tile([P, 8], F32, tag="proj")
                for k in range(KD):
                    nc.tensor.matmul(ps[:], wk[:, k, ts(j, P)], zb[:, k, :],
                                     start=(k == 0), stop=(k == KD - 1))
                for hh in (2 * j, 2 * j + 1):
                    r0 = (hh % 2) * 64
                    nc.vector.tensor_scalar(
                        kz[0:64, ts(hh, 8)], ps[r0:r0 + 64, :],
                        bias["cbk"][r0:r0 + 64, j, None], None, ALU.add)
            for hh in range(H):
                nc.vector.tensor_copy(kz[64:72, ts(hh, 8)], zhot[:])
            for ck, cw in ((0, 512), (512, 256)):
                ps = pmm.tile([8, 512], F32, tag="proj")
                for k in range(KD):
                    nc.tensor.matmul(ps[:, 0:cw], zb[:, k, :],
                                     wv[:, k, ck:ck + cw], start=(k == 0),
                                     stop=(k == KD - 1))
                h0, nh = ck // 64, cw // 64
                nc.vector.tensor_tensor(
                    vz[:, h0:h0 + nh, 0:64],
                    ps[:, 0:cw].rearrange("p (h d) -> p h d", d=64),
                    vb_c[0:8, 65 * h0:65 * (h0 + nh)].rearrange(
                        "p (h d) -> p h d", d=65)[:, :, 0:64],
                    ALU.add)
            nc.any.memset(vz[:, :, 64:65], 1.0)

            wq = wp.tile([P, KD, D], BF16, tag="w")
            nc.sync.dma_start(wq[:],
                              io["c_wq"].rearrange("(j p) o -> p j o", p=P))
            u1 = s1p.tile([P, KD, S], BF16, tag="u1")
            for j in range(KD):
                qa = {}
                for hh in (2 * j, 2 * j + 1):
                    qa[hh] = qcp.tile([72, S], BF16, tag="qc", name=f"qc{j}_{hh}")
                    nc.vector.tensor_copy(qa[hh][64:72, :], qmask[:])
                for c in range(NCH):
                    ps = pmm.tile([P, 512], F32, tag="proj")
                    for k in range(KD):
                        nc.tensor.matmul(ps[:], wq[:, k, ts(j, P)],
                                         xt[:, k, ts(c, 512)], start=(k == 0),
                                         stop=(k == KD - 1))
                    for hh in (2 * j, 2 * j + 1):
                        r0 = (hh % 2) * 64
                        nc.vector.tensor_scalar(
                            qa[hh][0:64, ts(c, 512)], ps[r0:r0 + 64, :],
                            bias["cbq"][r0:r0 + 64, j, None], None, ALU.add)
                for hh in (2 * j, 2 * j + 1):
                    for half in range(2):
                        ps = psc.tile([8, 2, 512], F32, tag="zsc")
                        for i in range(2):
                            c = half * 2 + i
                            nc.tensor.matmul(ps[:, i, :], kz[:, ts(hh, 8)],
                                             qa[hh][:, ts(c, 512)],
                                             start=True, stop=True)
                        e8 = tmp.tile([8, 2, 512], BF16, tag="e8")
                        nc.scalar.activation(e8[:], ps[:], AF.Exp)
                        for i in range(2):
                            c = half * 2 + i
                            ov = pav.tile([65, 512], F32, tag="zav")
                            nc.tensor.matmul(ov[:], vz[:, hh, :], e8[:, i, :],
                                             start=True, stop=True)
                            den = small.tile([1, 512], F32, tag="den")
                            nc.vector.reciprocal(den[:], ov[64:65, :])
                            db = tmp.tile([64, 512], F32, tag="db")
                            nc.gpsimd.partition_broadcast(db[:], den[:])
                            nc.vector.tensor_tensor(
                                u1[(hh % 2) * 64:(hh % 2) * 64 + 64, hh // 2,
                                   ts(c, 512)],
                                ov[0:64, :], db[:], ALU.mult)

            wc = wp.tile([P, KD, D], BF16, tag="w")
            nc.sync.dma_start(wc[:],
                              io["wc"].rearrange("(j p) o -> p j o", p=P))
            for j in range(KD):
                for c in range(NCH):
                    ps = pmm.tile([P, 512], F32, tag="proj")
                    for k in range(KD):
                        nc.tensor.matmul(ps[:], wc[:, k, ts(j, P)],
                                         u1[:, k, ts(c, 512)], start=(k == 0),
                                         stop=(k == KD - 1))
                    up = tmp.tile([P, 512], F32, tag="upd")
                    nc.scalar.activation(up[:], ps[:], AF.Identity,
                                         bias=bias["bc"][:, j, None])
                    dst = getx(j, c)
                    nc.vector.tensor_tensor(dst, dst, up[:], ALU.add)

        # ---------------- stage 2: self attention ----------------
        sc1_s = persist.tile([P, KD], F32, tag="sc1_s")
        nc.vector.tensor_scalar(sc1_s[:], modap(1), 1.0, None, ALU.add)
        _ln(tc, nc, st, getx, S, rb, mbb, ones_r, sc1_s, modap(0), xt)
        xst.close()  # x_rest dead: free 36KB/partition before attention

        with tc.tile_pool(name="s2w", bufs=2) as wp, \
                tc.tile_pool(name="s2", bufs=1) as s2p, \
                tc.tile_pool(name="s2k", bufs=3) as kqp, \
                tc.tile_pool(name="s2mm", bufs=2, space="PSUM") as pmm, \
                tc.tile_pool(name="s2sc", bufs=2, space="PSUM") as psc, \
                tc.tile_pool(name="s2av", bufs=2, space="PSUM") as pav:
            wv2 = wp.tile([P, KD, D], BF16, tag="w")
            nc.sync.dma_start(wv2[:],
                              io["s_wv"].rearrange("(j p) o -> p j o", p=P))
            vpad = s2p.tile([P, S // P, H * 65], BF16, tag="vpad")
            for i in range(S // P):
                for ck, cw in ((0, 512), (512, 256)):
                    ps = pmm.tile([P, 512], F32, tag="proj")
                    for k in range(KD):
                        nc.tensor.matmul(
                            ps[:, 0:cw],
                            xt[:, k, ts(i, P)], wv2[:, k, ck:ck + cw],
                            start=(k == 0), stop=(k == KD - 1))
                    h0, nh = ck // 64, cw // 64
                    nc.vector.tensor_tensor(
                        vpad[:, i, 65 * h0:65 * (h0 + nh)].rearrange(
                            "p (h d) -> p h d", d=65)[:, :, 0:64],
                        ps[:, 0:cw].rearrange("p (h d) -> p h d", d=64),
                        vb_s[:, 65 * h0:65 * (h0 + nh)].rearrange(
                            "p (h d) -> p h d", d=65)[:, :, 0:64],
                        ALU.add)
            nc.any.memset(
                vpad[:].rearrange("p i (h d) -> p i h d", d=65)[:, :, :,
                                                                64:65], 1.0)

            wq2 = wp.tile([P, KD, D], BF16, tag="w")
            nc.sync.dma_start(wq2[:],
                              io["s_wq"].rearrange("(j p) o -> p j o", p=P))
            wk2 = wp.tile([P, KD, D], BF16, tag="w")
            nc.sync.dma_start(wk2[:],
                              io["s_wk"].rearrange("(j p) o -> p j o", p=P))
            for j in range(KD):
                kpa, qa = {}, {}
                for hh in (2 * j, 2 * j + 1):
                    kpa[hh] = kqp.tile([72, S], BF16, tag="kpad", name=f"kp{j}_{hh}")
                    nc.vector.tensor_copy(kpa[hh][64:72, :], khot[:])
                    qa[hh] = kqp.tile([72, OWN], BF16, tag="q2a", name=f"q2{j}_{hh}")
                    nc.vector.tensor_copy(qa[hh][64:72, :], qmask[:, 0:OWN])
                for c in range(NCH):
                    ps = pmm.tile([P, 512], F32, tag="proj")
                    for k in range(KD):
                        nc.tensor.matmul(ps[:], wk2[:, k, ts(j, P)],
                                         xt[:, k, ts(c, 512)], start=(k == 0),
                                         stop=(k == KD - 1))
                    for hh in (2 * j, 2 * j + 1):
                        r0 = (hh % 2) * 64
                        nc.vector.tensor_scalar(
                            kpa[hh][0:64, ts(c, 512)], ps[r0:r0 + 64, :],
                            bias["sbk"][r0:r0 + 64, j, None], None, ALU.add)
                ps = pmm.tile([P, 512], F32, tag="proj")
                for k in range(KD):
                    nc.tensor.matmul(ps[:], wq2[:, k, ts(j, P)],
                                     xt[:, k, 0:OWN], start=(k == 0),
                                     stop=(k == KD - 1))
                for hh in (2 * j, 2 * j + 1):
                    r0 = (hh % 2) * 64
                    nc.vector.tensor_scalar(qa[hh][0:64, :], ps[r0:r0 + 64, :],
                                            bias["sbq"][r0:r0 + 64, j, None],
                                            None, ALU.add)
                for hh in (2 * j, 2 * j + 1):
                    ov = pav.tile([65, OWN], F32, tag="av")
                    for pair in range(S // 256):
                        ps2 = psc.tile([P, 2, OWN], F32, tag="sc")
                        for i in range(2):
                            kt = pair * 2 + i
                            nc.tensor.matmul(ps2[:, i, :],
                                             kpa[hh][:, ts(kt, P)],
                                             qa[hh][:], start=True, stop=True)
                        e = tmp.tile([P, 2, OWN], BF16, tag="e")
                        nc.scalar.activation(e[:], ps2[:], AF.Exp)
                        for i in range(2):
                            kt = pair * 2 + i
                            nc.tensor.matmul(
                                ov[:], vpad[:, kt, ts(hh, 65)], e[:, i, :],
                                start=(pair == 0 and i == 0),
                                stop=(pair == S // 256 - 1 and i == 1))
                    den = small.tile([1, OWN], F32, tag="den")
                    nc.vector.reciprocal(den[:], ov[64:65, :])
                    db = tmp.tile([64, OWN], F32, tag="db")
                    nc.gpsimd.partition_broadcast(db[:], den[:])
                    nc.vector.tensor_tensor(
                        u2[(hh % 2) * 64:(hh % 2) * 64 + 64, hh // 2, :],
                        ov[0:64, :], db[:], ALU.mult)

            ws = wp.tile([P, KD, D], BF16, tag="w")
            nc.sync.dma_start(ws[:],
                              io["ws"].rearrange("(j p) o -> p j o", p=P))
            for j in range(KD):
                ps = pmm.tile([P, 512], F32, tag="proj")
                for k in range(KD):
                    nc.tensor.matmul(ps[:], ws[:, k, ts(j, P)], u2[:, k, :],
                                     start=(k == 0), stop=(k == KD - 1))
                up = tmp.tile([P, OWN], F32, tag="upd")
                nc.vector.tensor_scalar(up[:], ps[:], bias["bs"][:, j, None],
                                        modap(2)[:, j, None], ALU.add,
                                        ALU.mult)
                nc.vector.tensor_tensor(x_own[:, j, :], x_own[:, j, :], up[:],
                                        ALU.add)

    # ---------------- stage 3: MLP (own tokens) ----------------
    sc1_m = persist.tile([P, KD], F32, tag="sc1_m")
    nc.vector.tensor_scalar(sc1_m[:], modap(4), 1.0, None, ALU.add)
    with tc.tile_pool(name="mlp", bufs=1) as mp, \
            tc.tile_pool(name="mmm", bufs=3, space="PSUM") as pmm:
        x3 = mp.tile([P, KD, OWN], BF16, tag="x3")
        _ln(tc, nc, st, lambda j, c: x_own[:, j, :], OWN, rb, mbb, ones_r,
            sc1_m, modap(3), x3)
        w1 = mp.tile([P, KD, DFF], BF16, tag="w1")
        nc.sync.dma_start(w1[:], io["m_w1"].rearrange("(j p) o -> p j o", p=P))
        h1 = mp.tile([P, KF, OWN], BF16, tag="h1")
        for j in range(KF):
            ps = pmm.tile([P, OWN], F32, tag="proj")
            for k in range(KD):
                nc.tensor.matmul(ps[:], w1[:, k, ts(j, P)], x3[:, k, :],
                                 start=(k == 0), stop=(k == KD - 1))
            nc.scalar.activation(h1[:, j, :], ps[:], AF.Gelu_apprx_tanh,
                                 bias=mb1[:, j, None])
        w2 = mp.tile([P, KF, D], BF16, tag="w2")
        nc.sync.dma_start(w2[:], io["m_w2"].rearrange("(j p) o -> p j o", p=P))
        for j in range(KD):
            ps = pmm.tile([P, OWN], F32, tag="proj")
            for k in range(KF):
                nc.tensor.matmul(ps[:], w2[:, k, ts(j, P)], h1[:, k, :],
                                 start=(k == 0), stop=(k == KF - 1))
            up = tmp.tile([P, OWN], F32, tag="upd")
            nc.vector.tensor_scalar(up[:], ps[:], bias["mb2"][:, j, None],
                                    modap(5)[:, j, None], ALU.add, ALU.mult)
            nc.vector.tensor_tensor(x_own[:, j, :], x_own[:, j, :], up[:],
                                    ALU.add)

    nc.sync.dma_start(io["xout"].rearrange("(j p) t -> p j t", p=P),
                      x_own[:])
    st.close()


def _build_nc():
    nc = bacc.Bacc("TRN2", target_bir_lowering=False, debug=False,
                   num_devices=NCORE)
    io = {}

    def din(name, shape, dt):
        io[name] = nc.dram_tensor(name, list(shape), dt,
                                  kind="ExternalInput").ap()

    din("xT", (D, S), F32)
    din("zT", (D, 8), BF16)
    din("tsil", (P, KD), BF16)
    for w in ("c_wq", "c_wk", "c_wv", "wc", "s_wq", "s_wk", "s_wv", "ws"):
        din(w, (D, D), BF16)
    din("m_w1", (D, DFF), BF16)
    din("m_w2", (DFF, D), BF16)
    din("w_ada", (D, 6 * D), BF16)
    din("bada", (P, 36), F32)
    for b in ("cbq", "cbk", "bc", "sbq", "sbk", "bs", "mb2"):
        din(b, (P, KD), F32)
    din("mb1", (P, KF), F32)
    din("cbv_row", (1, H * 65), BF16)
    din("sbv_row", (1, H * 65), BF16)
    din("qmask", (8, S), BF16)
    din("khot", (8, S), BF16)
    din("zhot", (8, 8), BF16)
    io["xout"] = nc.dram_tensor("xout", [D, OWN], F32,
                                kind="ExternalOutput").ap()

    with tile.TileContext(nc) as tc:
        _emit_kernel(tc, io)
    nc.compile()
    return nc


_NC_CACHE = {}
LAST_RESULTS = {}


def host_prep(inputs):
    ip = {k: np.asarray(v, np.float32) for k, v in inputs.items()
          if k != "n_frames"}
    sc = hd ** -0.5
    w = {}
    w["c_wq"] = (ip["c_wq"] * sc).astype(_bf)
    w["cbq_f"] = ip["c_bq"] * sc
    w["c_wk"] = ip["c_wk"].astype(_bf)
    w["cbk_f"] = ip["c_bk"]
    w["c_wv"] = ip["c_wv"].astype(_bf)
    w["cbv_f"] = ip["c_bv"]
    w["wc"] = (ip["c_wo"] @ ip["w_fc1"]).astype(_bf)
    w["bc_f"] = ip["c_bo"] @ ip["w_fc1"] + ip["b_fc1"]
    w["s_wq"] = (ip["s_wq"] * sc).astype(_bf)
    w["sbq_f"] = ip["s_bq"] * sc
    w["s_wk"] = ip["s_wk"].astype(_bf)
    w["sbk_f"] = ip["s_bk"]
    w["s_wv"] = ip["s_wv"].astype(_bf)
    w["sbv_f"] = ip["s_bv"]
    w["ws"] = (ip["s_wo"] @ ip["w_fc2"]).astype(_bf)
    w["bs_f"] = ip["s_bo"] @ ip["w_fc2"] + ip["b_fc2"]
    w["m_w1"] = ip["m_w1"].astype(_bf)
    w["mb1_f"] = ip["m_b1"]
    w["m_w2"] = ip["m_w2"].astype(_bf)
    w["mb2_f"] = ip["m_b2"]
    w["w_ada"] = ip["w_ada"].astype(_bf)
    w["bada_f"] = ip["b_ada"]
    return ip, w


def _ftile(v):
    """[n*128] -> [128, n] feature-tile layout (partition p, tile j) = v[128j+p]."""
    return np.ascontiguousarray(v.reshape(-1, P).T).astype(np.float32)


def _vrow(v):
    out = np.zeros((1, H * 65), np.float32)
    out[0].reshape(H, 65)[:, :64] = v.reshape(H, 64)
    return out


def core_in_map(c, ip, w):
    g, b = c % 4, c // 4
    fA, fB = g, 7 - g
    perm = [fA, fB] + [f for f in range(8) if f not in (fA, fB)]
    x = ip["x"]
    x_perm = np.concatenate([x[b * T + fr] for fr in perm], axis=0)
    frame_of = np.repeat(np.array(perm), NT)
    qmask = np.where(np.arange(8)[:, None] > frame_of[None, :], NEG,
                     0.0).astype(_bf)
    khot = (frame_of[None, :] == np.arange(8)[:, None]).astype(_bf)
    return {
        "xT": np.ascontiguousarray(x_perm.T),
        "zT": np.ascontiguousarray(ip["z"][b].T).astype(_bf),
        "tsil": _ftile(ip["t"][b]).astype(_bf),
        "c_wq": w["c_wq"], "c_wk": w["c_wk"], "c_wv": w["c_wv"],
        "wc": w["wc"], "s_wq": w["s_wq"], "s_wk": w["s_wk"],
        "s_wv": w["s_wv"], "ws": w["ws"], "m_w1": w["m_w1"],
        "m_w2": w["m_w2"], "w_ada": w["w_ada"],
        "bada": _ftile(w["bada_f"]).reshape(P, 36),
        "cbq": _ftile(w["cbq_f"]), "cbk": _ftile(w["cbk_f"]),
        "bc": _ftile(w["bc_f"]), "sbq": _ftile(w["sbq_f"]),
        "sbk": _ftile(w["sbk_f"]), "bs": _ftile(w["bs_f"]),
        "mb2": _ftile(w["mb2_f"]), "mb1": _ftile(w["mb1_f"]),
        "cbv_row": _vrow(w["cbv_f"]).astype(_bf), "sbv_row": _vrow(w["sbv_f"]).astype(_bf),
        "qmask": qmask, "khot": khot,
        "zhot": np.eye(8, dtype=np.float32).astype(_bf),
    }


def kernel(**inputs):
    ip, w = host_prep(inputs)
    in_maps = [core_in_map(c, ip, w) for c in range(NCORE)]
    if "nc" not in _NC_CACHE:
        _NC_CACHE["nc"] = _build_nc()
    nc = _NC_CACHE["nc"]
    res = run_bass_kernel_spmd(nc, in_maps, core_ids=list(range(NCORE)))
    LAST_RESULTS["res"] = res
    out = np.zeros((B * T, NT, D), np.float32)
    for c in range(NCORE):
        g, b = c % 4, c // 4
        fA, fB = g, 7 - g
        xo = res.results[c]["xout"]
        out[b * T + fA] = xo[:, :NT].T
        out[b * T + fB] = xo[:, NT:2 * NT].T
    return out


# revision 17
# speedup vs baseline: 1.4642x; 1.4642x over previous
"""Trainium2 Bass kernel for nn_CrossAttnVDTBlock (B=2,T=8,N=256,D=768,H=12,DFF=3072).

Sharding: 8 NeuronCores = 2 batch-groups x 4 frame-pair shards. Core c serves
batch c//4 and owns query frames (g, 7-g), g=c%4 (512 tokens, host-permuted to
the front). v1 is collective-free: each core redundantly computes the
cross-attention stage and the self-attention K/V for its whole batch (2048
tokens), then self-attention scores/AV and the MLP only for its own 512 query
tokens. Frame-causal masks are folded into the score matmuls via 8 augmented
contraction rows (K side: one-hot frame id; Q side: -30000*[f > frame(q)]), so
masking costs no elementwise work. The host pre-fuses c_wo@w_fc1 and s_wo@w_fc2
(no nonlinearity between them), folds hd^-0.5 into wq, and casts weights to
bf16. Matmuls run bf16 (fp32 PSUM); the residual stream stays fp32 on-chip.
Activations are feature-major [D, tokens] throughout - no device transposes.
"""

import contextlib

import numpy as np
import ml_dtypes

import concourse.bass as bass
import concourse.mybir as mybir
import concourse.tile as tile
from concourse import bacc
from concourse.bass import ts
from concourse.bass_utils import run_bass_kernel_spmd

F32 = mybir.dt.float32
F32R = mybir.dt.float32r
BF16 = mybir.dt.bfloat16
AF = mybir.ActivationFunctionType
ALU = mybir.AluOpType

B, T, NT, D, H, DFF = 2, 8, 256, 768, 12, 3072
hd = D // H          # 64
S = T * NT           # 2048
P = 128
KD = D // P          # 6 din tiles
KF = DFF // P        # 24 dff tiles
NEG = -30000.0
EPS = 1e-6
NCORE = 8
OWN = 512
NCH = S // 512       # 4 column chunks of 512

_bf = ml_dtypes.bfloat16


def _ln(tc, nc, ctx, getx, ncols, rb, mb, ones, sc1_ap, sh_ap, out_xt):
    """LayerNorm over features of feature-major x (via getx(j, chunk) -> AP
    [128,512]), optionally adaLN-modulated; writes bf16 out_xt [128,KD,ncols].
    rb/mb: [128,>=ncols] bf16 broadcast scratch."""
    nchunks = ncols // 512
    with tc.tile_pool(name="lnp", bufs=2, space="PSUM") as pp, \
            tc.tile_pool(name="lns", bufs=1) as sp, \
            tc.tile_pool(name="lnt", bufs=3) as tp:
        for c in range(nchunks):
            cs = ts(c, 512)
            ps_s = pp.tile([1, 512], F32, tag="ln_s")
            ps_q = pp.tile([1, 512], F32, tag="ln_q")
            for j in range(KD):
                xj = getx(j, c)
                xb = tp.tile([P, 512], BF16, tag="xb")
                nc.gpsimd.tensor_copy(xb[:], xj)
                xsq = tp.tile([P, 512], BF16, tag="xsq")
                nc.gpsimd.tensor_tensor(xsq[:], xj, xj, ALU.mult)
                nc.tensor.matmul(ps_s[:], ones[:], xb[:],
                                 start=(j == 0), stop=(j == KD - 1))
                nc.tensor.matmul(ps_q[:], ones[:], xsq[:],
                                 start=(j == 0), stop=(j == KD - 1))
            nc.vector.tensor_scalar_mul(ps_s[:], ps_s[:], -1.0 / D)
            nc.vector.tensor_scalar(ps_q[:], ps_q[:], 1.0 / D, EPS, ALU.mult,
                                    ALU.add)
            mu2 = sp.tile([1, 512], F32, tag="mu2", name=f"mu2_{c}")
            nc.scalar.activation(mu2[:], ps_s[:], AF.Square)
            nc.vector.tensor_tensor(ps_q[:], ps_q[:], mu2[:], ALU.subtract)
            nc.scalar.activation(ps_q[:], ps_q[:], AF.Sqrt)
            rr = sp.tile([1, 512], F32, tag="rr", name=f"rr_{c}")
            nc.vector.reciprocal(rr[:], ps_q[:])
            nm = sp.tile([1, 512], F32, tag="nm", name=f"nm_{c}")
            nc.vector.tensor_tensor(nm[:], ps_s[:], rr[:], ALU.mult)
            rrb = sp.tile([1, 512], BF16, tag="rrb", name=f"rrb_{c}")
            nc.vector.tensor_copy(rrb[:], rr[:])
            nmb = sp.tile([1, 512], BF16, tag="nmb", name=f"nmb_{c}")
            nc.vector.tensor_copy(nmb[:], nm[:])
            nc.gpsimd.partition_broadcast(rb[:, cs], rrb[:])
            nc.gpsimd.partition_broadcast(mb[:, cs], nmb[:])
        for j in range(KD):
            for c in range(nchunks):
                cs = ts(c, 512)
                t1 = tp.tile([P, 512], F32, tag="lnt1")
                nc.vector.tensor_tensor(t1[:], getx(j, c), rb[:, cs], ALU.mult)
                if sc1_ap is None:
                    nc.vector.tensor_tensor(out_xt[:, j, cs], t1[:], mb[:, cs],
                                            ALU.add)
                else:
                    nc.vector.tensor_tensor(t1[:], t1[:], mb[:, cs], ALU.add)
                    nc.vector.tensor_scalar(out_xt[:, j, cs], t1[:],
                                            sc1_ap[:, j, None],
                                            sh_ap[:, j, None],
                                            ALU.mult, ALU.add)


def _emit_kernel(tc, io):
    nc = tc.nc
    st = contextlib.ExitStack()
    pool = lambda **kw: st.enter_context(tc.tile_pool(**kw))

    persist = pool(name="persist", bufs=1)
    tmp = pool(name="tmp", bufs=2)
    small = pool(name="small", bufs=2)

    # ---------------- persistent state ----------------
    x_own = persist.tile([P, KD, OWN], F32, tag="x_own")
    ones_r = persist.tile([P, 1], BF16, tag="ones")
    nc.any.memset(ones_r[:], 1.0)
    rb = persist.tile([P, S], BF16, tag="rb")
    mbb = persist.tile([P, S], BF16, tag="mbb")
    mods = persist.tile([P, 36], F32, tag="mods")
    qmask = persist.tile([8, S], BF16, tag="qmask")
    khot = persist.tile([8, S], BF16, tag="khot")
    zhot = persist.tile([8, 8], BF16, tag="zhot")
    zb = persist.tile([P, KD, 8], BF16, tag="zb")
    u2 = persist.tile([P, KD, OWN], BF16, tag="u2")

    nc.sync.dma_start(x_own[:],
                      io["xT"][:, 0:OWN].rearrange("(j p) t -> p j t", p=P))
    nc.sync.dma_start(qmask[:], io["qmask"][:])
    nc.sync.dma_start(khot[:], io["khot"][:])
    nc.sync.dma_start(zhot[:], io["zhot"][:])
    nc.sync.dma_start(zb[:], io["zT"].rearrange("(j p) t -> p j t", p=P))

    bias = {}
    for nm_ in ("cbq", "cbk", "bc", "sbq", "sbk", "bs", "mb2"):
        bt = persist.tile([P, KD], F32, tag="b_" + nm_)
        nc.sync.dma_start(bt[:], io[nm_][:])
        bias[nm_] = bt
    mb1 = persist.tile([P, KF], F32, tag="b_mb1")
    nc.sync.dma_start(mb1[:], io["mb1"][:])
    vrow_c = persist.tile([1, H * 65], BF16, tag="vrow_c")
    nc.sync.dma_start(vrow_c[:], io["cbv_row"][:])
    vrow_s = persist.tile([1, H * 65], BF16, tag="vrow_s")
    nc.sync.dma_start(vrow_s[:], io["sbv_row"][:])
    vb_c = persist.tile([P, H * 65], BF16, tag="vb_c")
    nc.gpsimd.partition_broadcast(vb_c[:], vrow_c[:])
    vb_s = persist.tile([P, H * 65], BF16, tag="vb_s")
    nc.gpsimd.partition_broadcast(vb_s[:], vrow_s[:])

    def modap(i):  # chunk i of (sh_s, sc_s, g_s, sh_m, sc_m, g_m)
        return mods[:, ts(i, KD)]

    # ---------------- mods = silu(t) @ w_ada + b_ada ----------------
    with tc.tile_pool(name="ada", bufs=1) as ap, \
            tc.tile_pool(name="adap", bufs=1, space="PSUM") as app:
        wada = ap.tile([P, KD, 6 * D], BF16)
        nc.sync.dma_start(wada[:],
                          io["w_ada"].rearrange("(j p) o -> p j o", p=P))
        tsil = ap.tile([P, KD], BF16)
        nc.sync.dma_start(tsil[:], io["tsil"][:])
        tsig = ap.tile([P, KD], BF16)
        nc.scalar.activation(tsig[:], tsil[:], AF.Silu)
        bada = ap.tile([P, 36], F32)
        nc.sync.dma_start(bada[:], io["bada"][:])
        ps_m = app.tile([P, 36], F32)
        for m in range(36):
            for j in range(KD):
                nc.tensor.matmul(ps_m[:, m:m + 1], wada[:, j, ts(m, P)],
                                 tsig[:, j, None], start=(j == 0),
                                 stop=(j == KD - 1))
        nc.vector.tensor_tensor(mods[:], ps_m[:], bada[:], ALU.add)

    # =========== stages 1+2 need the full-batch residual ===========
    with tc.tile_pool(name="bigx", bufs=1) as bigp:
        xst = contextlib.ExitStack()
        xrp = xst.enter_context(tc.tile_pool(name="xrestp", bufs=1))
        x_rest = xrp.tile([P, KD, S - OWN], F32, tag="x_rest")
        nc.sync.dma_start(
            x_rest[:], io["xT"][:, OWN:S].rearrange("(j p) t -> p j t", p=P))
        xt = bigp.tile([P, KD, S], BF16, tag="xt")  # normalized activations

        def getx(j, c):
            if c == 0:
                return x_own[:, j, :]
            return x_rest[:, j, ts(c - 1, 512)]

        # ---------------- stage 1: cross attention ----------------
        _ln(tc, nc, st, getx, S, rb, mbb, ones_r, None, None, xt)

        with tc.tile_pool(name="s1w", bufs=2) as wp, \
                tc.tile_pool(name="s1", bufs=1) as s1p, \
                tc.tile_pool(name="s1q", bufs=2) as qcp, \
                tc.tile_pool(name="s1mm", bufs=2, space="PSUM") as pmm, \
                tc.tile_pool(name="s1sc", bufs=2, space="PSUM") as psc, \
                tc.tile_pool(name="s1av", bufs=2, space="PSUM") as pav:
            wk = wp.tile([P, KD, D], BF16, tag="w")
            nc.sync.dma_start(wk[:],
                              io["c_wk"].rearrange("(j p) o -> p j o", p=P))
            wv = wp.tile([P, KD, D], BF16, tag="w")
            nc.sync.dma_start(wv[:],
                              io["c_wv"].rearrange("(j p) o -> p j o", p=P))
            # kz feature-major [72, 8] per head; vz grouped [32, 3, 4, 65]
            kz = s1p.tile([72, 8 * H], BF16, tag="kz")
            vz = s1p.tile([8, H, 65], BF16, tag="vz")
            for j in range(KD):
                ps = pmm.tile([P, 8], F32, tag="proj")
                for k in range(KD):
                    nc.tensor.matmul(ps[:], wk[:, k, ts(j, P)], zb[:, k, :],
                                     start=(k == 0), stop=(k == KD - 1))
                for hh in (2 * j, 2 * j + 1):
                    r0 = (hh % 2) * 64
                    nc.scalar.activation(
                        kz[0:64, ts(hh, 8)], ps[r0:r0 + 64, :], AF.Identity,
                        bias=bias["cbk"][r0:r0 + 64, j, None])
            for hh in range(H):
                nc.vector.tensor_copy(kz[64:72, ts(hh, 8)], zhot[:])
            for ck, cw in ((0, 512), (512, 256)):
                ps = pmm.tile([8, 512], F32, tag="proj")
                for k in range(KD):
                    nc.tensor.matmul(ps[:, 0:cw], zb[:, k, :],
                                     wv[:, k, ck:ck + cw], start=(k == 0),
                                     stop=(k == KD - 1))
                h0, nh = ck // 64, cw // 64
                nc.vector.tensor_tensor(
                    vz[:, h0:h0 + nh, 0:64],
                    ps[:, 0:cw].rearrange("p (h d) -> p h d", d=64),
                    vb_c[0:8, 65 * h0:65 * (h0 + nh)].rearrange(
                        "p (h d) -> p h d", d=65)[:, :, 0:64],
                    ALU.add)
            nc.any.memset(vz[:, :, 64:65], 1.0)

            wq = wp.tile([P, KD, D], BF16, tag="w")
            nc.sync.dma_start(wq[:],
                              io["c_wq"].rearrange("(j p) o -> p j o", p=P))
            u1 = s1p.tile([P, KD, S], BF16, tag="u1")
            for j in range(KD):
                qa = {}
                for hh in (2 * j, 2 * j + 1):
                    qa[hh] = qcp.tile([72, S], BF16, tag="qc", name=f"qc{j}_{hh}")
                    nc.vector.tensor_copy(qa[hh][64:72, :], qmask[:])
                for c in range(NCH):
                    ps = pmm.tile([P, 512], F32, tag="proj")
                    for k in range(KD):
                        nc.tensor.matmul(ps[:], wq[:, k, ts(j, P)],
                                         xt[:, k, ts(c, 512)], start=(k == 0),
                                         stop=(k == KD - 1))
                    for hh in (2 * j, 2 * j + 1):
                        r0 = (hh % 2) * 64
                        nc.scalar.activation(
                            qa[hh][0:64, ts(c, 512)], ps[r0:r0 + 64, :],
                            AF.Identity,
                            bias=bias["cbq"][r0:r0 + 64, j, None])
                for hh in (2 * j, 2 * j + 1):
                    for half in range(2):
                        ps = psc.tile([8, 2, 512], F32, tag="zsc")
                        for i in range(2):
                            c = half * 2 + i
                            nc.tensor.matmul(ps[:, i, :], kz[:, ts(hh, 8)],
                                             qa[hh][:, ts(c, 512)],
                                             start=True, stop=True)
                        e8 = tmp.tile([8, 2, 512], BF16, tag="e8")
                        nc.scalar.activation(e8[:], ps[:], AF.Exp)
                        for i in range(2):
                            c = half * 2 + i
                            ov = pav.tile([65, 512], F32, tag="zav")
                            nc.tensor.matmul(ov[:], vz[:, hh, :], e8[:, i, :],
                                             start=True, stop=True)
                            den = small.tile([1, 512], F32, tag="den")
                            nc.vector.reciprocal(den[:], ov[64:65, :])
                            db = tmp.tile([64, 512], F32, tag="db")
                            nc.gpsimd.partition_broadcast(db[:], den[:])
                            nc.vector.tensor_tensor(
                                u1[(hh % 2) * 64:(hh % 2) * 64 + 64, hh // 2,
                                   ts(c, 512)],
                                ov[0:64, :], db[:], ALU.mult)

            wc = wp.tile([P, KD, D], BF16, tag="w")
            nc.sync.dma_start(wc[:],
                              io["wc"].rearrange("(j p) o -> p j o", p=P))
            for j in range(KD):
                for c in range(NCH):
                    ps = pmm.tile([P, 512], F32, tag="proj")
                    for k in range(KD):
                        nc.tensor.matmul(ps[:], wc[:, k, ts(j, P)],
                                         u1[:, k, ts(c, 512)], start=(k == 0),
                                         stop=(k == KD - 1))
                    up = tmp.tile([P, 512], F32, tag="upd")
                    nc.scalar.activation(up[:], ps[:], AF.Identity,
                                         bias=bias["bc"][:, j, None])
                    dst = getx(j, c)
                    nc.vector.tensor_tensor(dst, dst, up[:], ALU.add)

        # ---------------- stage 2: self attention ----------------
        sc1_s = persist.tile([P, KD], F32, tag="sc1_s")
        nc.vector.tensor_scalar(sc1_s[:], modap(1), 1.0, None, ALU.add)
        _ln(tc, nc, st, getx, S, rb, mbb, ones_r, sc1_s, modap(0), xt)
        xst.close()  # x_rest dead: free 36KB/partition before attention

        with tc.tile_pool(name="s2w", bufs=2) as wp, \
                tc.tile_pool(name="s2", bufs=1) as s2p, \
                tc.tile_pool(name="s2k", bufs=3) as kqp, \
                tc.tile_pool(name="s2mm", bufs=2, space="PSUM") as pmm, \
                tc.tile_pool(name="s2sc", bufs=2, space="PSUM") as psc, \
                tc.tile_pool(name="s2av", bufs=2, space="PSUM") as pav:
            wv2 = wp.tile([P, KD, D], BF16, tag="w")
            nc.sync.dma_start(wv2[:],
                              io["s_wv"].rearrange("(j p) o -> p j o", p=P))
            vpad = s2p.tile([P, S // P, H * 65], BF16, tag="vpad")
            for i in range(S // P):
                for ck, cw in ((0, 512), (512, 256)):
                    ps = pmm.tile([P, 512], F32, tag="proj")
                    for k in range(KD):
                        nc.tensor.matmul(
                            ps[:, 0:cw],
                            xt[:, k, ts(i, P)], wv2[:, k, ck:ck + cw],
                            start=(k == 0), stop=(k == KD - 1))
                    h0, nh = ck // 64, cw // 64
                    nc.vector.tensor_tensor(
                        vpad[:, i, 65 * h0:65 * (h0 + nh)].rearrange(
                            "p (h d) -> p h d", d=65)[:, :, 0:64],
                        ps[:, 0:cw].rearrange("p (h d) -> p h d", d=64),
                        vb_s[:, 65 * h0:65 * (h0 + nh)].rearrange(
                            "p (h d) -> p h d", d=65)[:, :, 0:64],
                        ALU.add)
            nc.any.memset(
                vpad[:].rearrange("p i (h d) -> p i h d", d=65)[:, :, :,
                                                                64:65], 1.0)

            wq2 = wp.tile([P, KD, D], BF16, tag="w")
            nc.sync.dma_start(wq2[:],
                              io["s_wq"].rearrange("(j p) o -> p j o", p=P))
            wk2 = wp.tile([P, KD, D], BF16, tag="w")
            nc.sync.dma_start(wk2[:],
                              io["s_wk"].rearrange("(j p) o -> p j o", p=P))
            for j in range(KD):
                kpa, qa = {}, {}
                for hh in (2 * j, 2 * j + 1):
                    kpa[hh] = kqp.tile([72, S], BF16, tag="kpad", name=f"kp{j}_{hh}")
                    nc.vector.tensor_copy(kpa[hh][64:72, :], khot[:])
                    qa[hh] = kqp.tile([72, OWN], BF16, tag="q2a", name=f"q2{j}_{hh}")
                    nc.vector.tensor_copy(qa[hh][64:72, :], qmask[:, 0:OWN])
                for c in range(NCH):
                    ps = pmm.tile([P, 512], F32, tag="proj")
                    for k in range(KD):
                        nc.tensor.matmul(ps[:], wk2[:, k, ts(j, P)],
                                         xt[:, k, ts(c, 512)], start=(k == 0),
                                         stop=(k == KD - 1))
                    for hh in (2 * j, 2 * j + 1):
                        r0 = (hh % 2) * 64
                        nc.scalar.activation(
                            kpa[hh][0:64, ts(c, 512)], ps[r0:r0 + 64, :],
                            AF.Identity,
                            bias=bias["sbk"][r0:r0 + 64, j, None])
                ps = pmm.tile([P, 512], F32, tag="proj")
                for k in range(KD):
                    nc.tensor.matmul(ps[:], wq2[:, k, ts(j, P)],
                                     xt[:, k, 0:OWN], start=(k == 0),
                                     stop=(k == KD - 1))
                for hh in (2 * j, 2 * j + 1):
                    r0 = (hh % 2) * 64
                    nc.scalar.activation(qa[hh][0:64, :], ps[r0:r0 + 64, :],
                                         AF.Identity,
                                         bias=bias["sbq"][r0:r0 + 64, j, None])
                # Prefix-K: query half A (own frame g<=3) only attends
                # frames <= 3, which in perm order live in ktiles
                # {0,1} u {4..9}; half B (frame 7-g) needs all 16. The aug
                # rows still mask the overreach exactly.
                A_KT = [0, 1, 4, 5, 6, 7, 8, 9]
                for hh in (2 * j, 2 * j + 1):
                    ov = pav.tile([65, OWN], F32, tag="av")
                    for half, kts in ((0, A_KT), (1, list(range(16)))):
                        qs = ts(half, 256)
                        n = len(kts)
                        for pp in range(n // 2):
                            ps2 = psc.tile([P, 2, 256], F32, tag="sc")
                            for i in range(2):
                                kt = kts[pp * 2 + i]
                                nc.tensor.matmul(ps2[:, i, :],
                                                 kpa[hh][:, ts(kt, P)],
                                                 qa[hh][:, qs], start=True,
                                                 stop=True)
                            e = tmp.tile([P, 2, 256], BF16, tag="e")
                            nc.scalar.activation(e[:], ps2[:], AF.Exp)
                            for i in range(2):
                                kt = kts[pp * 2 + i]
                                nc.tensor.matmul(
                                    ov[:, qs], vpad[:, kt, ts(hh, 65)],
                                    e[:, i, :],
                                    start=(pp == 0 and i == 0),
                                    stop=(pp == n // 2 - 1 and i == 1))
                        den = small.tile([1, 256], F32, tag="den",
                                         name=f"den{j}_{hh}_{half}")
                        nc.vector.reciprocal(den[:], ov[64:65, qs])
                        db = tmp.tile([64, 256], F32, tag="db")
                        nc.gpsimd.partition_broadcast(db[:], den[:])
                        nc.vector.tensor_tensor(
                            u2[(hh % 2) * 64:(hh % 2) * 64 + 64, hh // 2, qs],
                            ov[0:64, qs], db[:], ALU.mult)

            ws = wp.tile([P, KD, D], BF16, tag="w")
            nc.sync.dma_start(ws[:],
                              io["ws"].rearrange("(j p) o -> p j o", p=P))
            for j in range(KD):
                ps = pmm.tile([P, 512], F32, tag="proj")
                for k in range(KD):
                    nc.tensor.matmul(ps[:], ws[:, k, ts(j, P)], u2[:, k, :],
                                     start=(k == 0), stop=(k == KD - 1))
                up = tmp.tile([P, OWN], F32, tag="upd")
                nc.vector.tensor_scalar(up[:], ps[:], bias["bs"][:, j, None],
                                        modap(2)[:, j, None], ALU.add,
                                        ALU.mult)
                nc.vector.tensor_tensor(x_own[:, j, :], x_own[:, j, :], up[:],
                                        ALU.add)

    # ---------------- stage 3: MLP (own tokens) ----------------
    sc1_m = persist.tile([P, KD], F32, tag="sc1_m")
    nc.vector.tensor_scalar(sc1_m[:], modap(4), 1.0, None, ALU.add)
    with tc.tile_pool(name="mlp", bufs=1) as mp, \
            tc.tile_pool(name="mmm", bufs=3, space="PSUM") as pmm:
        x3 = mp.tile([P, KD, OWN], BF16, tag="x3")
        _ln(tc, nc, st, lambda j, c: x_own[:, j, :], OWN, rb, mbb, ones_r,
            sc1_m, modap(3), x3)
        w1 = mp.tile([P, KD, DFF], BF16, tag="w1")
        nc.sync.dma_start(w1[:], io["m_w1"].rearrange("(j p) o -> p j o", p=P))
        h1 = mp.tile([P, KF, OWN], BF16, tag="h1")
        for j in range(KF):
            ps = pmm.tile([P, OWN], F32, tag="proj")
            for k in range(KD):
                nc.tensor.matmul(ps[:], w1[:, k, ts(j, P)], x3[:, k, :],
                                 start=(k == 0), stop=(k == KD - 1))
            nc.scalar.activation(h1[:, j, :], ps[:], AF.Gelu_apprx_tanh,
                                 bias=mb1[:, j, None])
        w2 = mp.tile([P, KF, D], BF16, tag="w2")
        nc.sync.dma_start(w2[:], io["m_w2"].rearrange("(j p) o -> p j o", p=P))
        for j in range(KD):
            ps = pmm.tile([P, OWN], F32, tag="proj")
            for k in range(KF):
                nc.tensor.matmul(ps[:], w2[:, k, ts(j, P)], h1[:, k, :],
                                 start=(k == 0), stop=(k == KF - 1))
            up = tmp.tile([P, OWN], F32, tag="upd")
            nc.vector.tensor_scalar(up[:], ps[:], bias["mb2"][:, j, None],
                                    modap(5)[:, j, None], ALU.add, ALU.mult)
            nc.vector.tensor_tensor(x_own[:, j, :], x_own[:, j, :], up[:],
                                    ALU.add)

    nc.sync.dma_start(io["xout"].rearrange("(j p) t -> p j t", p=P),
                      x_own[:])
    st.close()


def _build_nc():
    nc = bacc.Bacc("TRN2", target_bir_lowering=False, debug=False,
                   num_devices=NCORE)
    io = {}

    def din(name, shape, dt):
        io[name] = nc.dram_tensor(name, list(shape), dt,
                                  kind="ExternalInput").ap()

    din("xT", (D, S), F32)
    din("zT", (D, 8), BF16)
    din("tsil", (P, KD), BF16)
    for w in ("c_wq", "c_wk", "c_wv", "wc", "s_wq", "s_wk", "s_wv", "ws"):
        din(w, (D, D), BF16)
    din("m_w1", (D, DFF), BF16)
    din("m_w2", (DFF, D), BF16)
    din("w_ada", (D, 6 * D), BF16)
    din("bada", (P, 36), F32)
    for b in ("cbq", "cbk", "bc", "sbq", "sbk", "bs", "mb2"):
        din(b, (P, KD), F32)
    din("mb1", (P, KF), F32)
    din("cbv_row", (1, H * 65), BF16)
    din("sbv_row", (1, H * 65), BF16)
    din("qmask", (8, S), BF16)
    din("khot", (8, S), BF16)
    din("zhot", (8, 8), BF16)
    io["xout"] = nc.dram_tensor("xout", [D, OWN], F32,
                                kind="ExternalOutput").ap()

    with tile.TileContext(nc) as tc:
        _emit_kernel(tc, io)
    nc.compile()
    return nc


_NC_CACHE = {}
LAST_RESULTS = {}


def host_prep(inputs):
    ip = {k: np.asarray(v, np.float32) for k, v in inputs.items()
          if k != "n_frames"}
    sc = hd ** -0.5
    w = {}
    w["c_wq"] = (ip["c_wq"] * sc).astype(_bf)
    w["cbq_f"] = ip["c_bq"] * sc
    w["c_wk"] = ip["c_wk"].astype(_bf)
    w["cbk_f"] = ip["c_bk"]
    w["c_wv"] = ip["c_wv"].astype(_bf)
    w["cbv_f"] = ip["c_bv"]
    w["wc"] = (ip["c_wo"] @ ip["w_fc1"]).astype(_bf)
    w["bc_f"] = ip["c_bo"] @ ip["w_fc1"] + ip["b_fc1"]
    w["s_wq"] = (ip["s_wq"] * sc).astype(_bf)
    w["sbq_f"] = ip["s_bq"] * sc
    w["s_wk"] = ip["s_wk"].astype(_bf)
    w["sbk_f"] = ip["s_bk"]
    w["s_wv"] = ip["s_wv"].astype(_bf)
    w["sbv_f"] = ip["s_bv"]
    w["ws"] = (ip["s_wo"] @ ip["w_fc2"]).astype(_bf)
    w["bs_f"] = ip["s_bo"] @ ip["w_fc2"] + ip["b_fc2"]
    w["m_w1"] = ip["m_w1"].astype(_bf)
    w["mb1_f"] = ip["m_b1"]
    w["m_w2"] = ip["m_w2"].astype(_bf)
    w["mb2_f"] = ip["m_b2"]
    w["w_ada"] = ip["w_ada"].astype(_bf)
    w["bada_f"] = ip["b_ada"]
    return ip, w


def _ftile(v):
    """[n*128] -> [128, n] feature-tile layout (partition p, tile j) = v[128j+p]."""
    return np.ascontiguousarray(v.reshape(-1, P).T).astype(np.float32)


def _vrow(v):
    out = np.zeros((1, H * 65), np.float32)
    out[0].reshape(H, 65)[:, :64] = v.reshape(H, 64)
    return out


def core_in_map(c, ip, w):
    g, b = c % 4, c // 4
    fA, fB = g, 7 - g
    perm = [fA, fB] + [f for f in range(8) if f not in (fA, fB)]
    x = ip["x"]
    x_perm = np.concatenate([x[b * T + fr] for fr in perm], axis=0)
    frame_of = np.repeat(np.array(perm), NT)
    qmask = np.where(np.arange(8)[:, None] > frame_of[None, :], NEG,
                     0.0).astype(_bf)
    khot = (frame_of[None, :] == np.arange(8)[:, None]).astype(_bf)
    return {
        "xT": np.ascontiguousarray(x_perm.T),
        "zT": np.ascontiguousarray(ip["z"][b].T).astype(_bf),
        "tsil": _ftile(ip["t"][b]).astype(_bf),
        "c_wq": w["c_wq"], "c_wk": w["c_wk"], "c_wv": w["c_wv"],
        "wc": w["wc"], "s_wq": w["s_wq"], "s_wk": w["s_wk"],
        "s_wv": w["s_wv"], "ws": w["ws"], "m_w1": w["m_w1"],
        "m_w2": w["m_w2"], "w_ada": w["w_ada"],
        "bada": _ftile(w["bada_f"]).reshape(P, 36),
        "cbq": _ftile(w["cbq_f"]), "cbk": _ftile(w["cbk_f"]),
        "bc": _ftile(w["bc_f"]), "sbq": _ftile(w["sbq_f"]),
        "sbk": _ftile(w["sbk_f"]), "bs": _ftile(w["bs_f"]),
        "mb2": _ftile(w["mb2_f"]), "mb1": _ftile(w["mb1_f"]),
        "cbv_row": _vrow(w["cbv_f"]).astype(_bf), "sbv_row": _vrow(w["sbv_f"]).astype(_bf),
        "qmask": qmask, "khot": khot,
        "zhot": np.eye(8, dtype=np.float32).astype(_bf),
    }


def kernel(**inputs):
    ip, w = host_prep(inputs)
    in_maps = [core_in_map(c, ip, w) for c in range(NCORE)]
    if "nc" not in _NC_CACHE:
        _NC_CACHE["nc"] = _build_nc()
    nc = _NC_CACHE["nc"]
    res = run_bass_kernel_spmd(nc, in_maps, core_ids=list(range(NCORE)))
    LAST_RESULTS["res"] = res
    out = np.zeros((B * T, NT, D), np.float32)
    for c in range(NCORE):
        g, b = c % 4, c // 4
        fA, fB = g, 7 - g
        xo = res.results[c]["xout"]
        out[b * T + fA] = xo[:, :NT].T
        out[b * T + fB] = xo[:, NT:2 * NT].T
    return out
